# revision 3
# baseline (speedup 1.0000x reference)
"""Trainium2 Bass kernel for nn_AdditiveAttention (B=8, NQ=NK=1024, D=256, H=64).

Strategy
--------
Data-parallel over batch: core b owns batch element b (no collectives).

The reference computes  scores[q,k] = sum_h Wv[h] * tanh(Qh[q,h] + Kh[k,h])
with Qh = Q@Wq, Kh = K@Wk, then a masked softmax over k and attn @ V.
A direct tanh evaluation needs NQ*NK*H = 67M ScalarE ops per core (~450us).

Instead we use an optimized sine expansion (odd function):
    tanh(x) ~= sum_m b_m sin(w_m x),   max err 8e-4 on |x| <= 10.35
and the angle-addition identity
    sin(w(a+b)) = sin(wa)cos(wb) + cos(wa)sin(wb)
which turns the score computation into M rank-128 TensorE contractions:
    scores = sum_m F_m^T G_m-style matmuls
with per-side features F_m = [sin(w_m Qh); cos(w_m Qh)] (128 x NQ) and
G_m = b_m*[Wv;Wv] * [cos(w_m Kh); sin(w_m Kh)] (128 x KL).

The device Sin activation is only valid on [-pi, pi], so each feature
argument is range-reduced with the fp32 magic-constant rounding trick:
    u = x*(w/2pi) + phase;  k = round(u) via +/-1.5*2^23;  r = u - k
    feature = sin(2pi r) = sin(w x + 2pi phase)
(the stt op produces -r for both sides; the signs cancel in the products).

Masking: columns k >= valid_len get exp(scores - 40) ~= 0 via a per-partition
bias on the Exp activation (k lives on partitions).  Only ceil(max(vl)/128)
k-chunks are computed at all.  The ones-column appended to V yields the
softmax denominator from the same matmul that computes attn @ V.
"""

import numpy as np

B, NQ, NK, DQ, DK, DV, H = 8, 1024, 1024, 256, 256, 256, 64
PI = float(np.pi)
MAGIC = float(1.5 * 2 ** 23)
MASK_NEG = -40.0

# M=10 frequency-optimized sine fit of tanh on [-10.35, 10.35]; max err 8.0e-4.
FIT_BOUND = 10.35
OMEGAS = [0.251318817, 0.746061764, 1.06480279, 1.33678918, 1.8444882,
          2.382742, 2.93435819, 3.49596816, 4.06399384, 4.62393325]
COEFS = [1.23949168, 0.324806206, 0.0559147295, 0.114892657, 0.0585742911,
         0.0258680335, 0.0110960367, 0.00466259993, 0.00191840823,
         0.000739195029]

_BUILD_CACHE: dict = {}


def _harmonic_fallback_fit(bound: float):
    """Numpy-only harmonic LSQ fit of tanh on [0, bound] (robustness path for
    inputs outside the precomputed fit domain)."""
    M, L = 24, bound * 1.16
    x = np.linspace(0, bound, 6001)
    om = np.arange(1, M + 1) * np.pi / L
    A = np.sin(np.outer(x, om))
    b, *_ = np.linalg.lstsq(A, np.tanh(x), rcond=None)
    return [float(v) for v in om], [float(v) for v in b]


def _build(kmax: int, omegas, coefs):
    import concourse.bacc as bacc
    import concourse.tile as tile
    from concourse import mybir

    AF = mybir.ActivationFunctionType
    ALU = mybir.AluOpType
    f32 = mybir.dt.float32
    bf16 = mybir.dt.bfloat16

    M = len(omegas)
    KL = kmax * 128
    # blob_a column layout (all f32, 128 partitions)
    #  [wq2 256 | wk2 256 | ph_q 1 | ph_k 1 | bwv 1 | maskb kmax | qtb 2048 | ktb 2*KL]
    WA = 512 + 3 + kmax + 2048 + 2 * KL
    WB = kmax * 260

    nc = bacc.Bacc("TRN2", target_bir_lowering=False, debug=False, num_devices=8)
    blob_a = nc.declare_dram_parameter("blob_a", [128, WA], f32, isOutput=False)
    blob_b = nc.declare_dram_parameter("blob_b", [128, WB], f32, isOutput=False)
    out_d = nc.declare_dram_parameter("out", [128, 8 * 256], f32, isOutput=True)

    with tile.TileContext(nc) as tc:
        with (
            tc.tile_pool(name="io", bufs=1) as pool_io,
            tc.tile_pool(name="proj_sb", bufs=1) as pool_proj,
            tc.tile_pool(name="scratch", bufs=2) as pool_scr,
            tc.tile_pool(name="feat", bufs=1) as pool_feat,
            tc.tile_pool(name="epool", bufs=1) as pool_e,
            tc.tile_pool(name="opool", bufs=1) as pool_o,
        ):
            stage_a = pool_io.tile([128, WA], f32, tag="stage_a")
            nc.gpsimd.dma_start(out=stage_a[:], in_=blob_a[:])
            stage_b = pool_io.tile([128, WB], f32, tag="stage_b")
            nc.gpsimd.dma_start(out=stage_b[:], in_=blob_b[:])

            wq2 = [stage_a[:, 0:128], stage_a[:, 128:256]]
            wk2 = [stage_a[:, 256:384], stage_a[:, 384:512]]
            ph_q = stage_a[:, 512:513]
            ph_k = stage_a[:, 513:514]
            bwv = stage_a[:, 514:515]
            mb0 = 515
            qt0 = 515 + kmax
            kt0 = qt0 + 2048
            qtv = [stage_a[:, qt0:qt0 + 1024], stage_a[:, qt0 + 1024:qt0 + 2048]]
            ktv = [stage_a[:, kt0:kt0 + KL], stage_a[:, kt0 + KL:kt0 + 2 * KL]]

            # --- projections: Qh2 = [Wq|Wq]^T Q^T (128 x 1024), same for K ---
            with tc.tile_pool(name="ps_proj", bufs=1, space="PSUM") as ps_proj:
                qh2_ps = ps_proj.tile([128, 1024], f32, tag="qh2")
                for s in range(2):  # f32 moving limit 512
                    sl = slice(s * 512, (s + 1) * 512)
                    nc.tensor.matmul(qh2_ps[:, sl], wq2[0], qtv[0][:, sl],
                                     start=True, stop=False)
                    nc.tensor.matmul(qh2_ps[:, sl], wq2[1], qtv[1][:, sl],
                                     start=False, stop=True)
                kh2_ps = ps_proj.tile([128, KL], f32, tag="kh2")
                nks = (KL + 511) // 512
                for s in range(nks):
                    sl = slice(s * 512, min((s + 1) * 512, KL))
                    nc.tensor.matmul(kh2_ps[:, sl], wk2[0], ktv[0][:, sl],
                                     start=True, stop=False)
                    nc.tensor.matmul(kh2_ps[:, sl], wk2[1], ktv[1][:, sl],
                                     start=False, stop=True)

                qh2 = pool_proj.tile([128, 1024], f32, tag="qh2sb")
                nc.vector.tensor_copy(qh2[:], qh2_ps[:])
                kh2 = pool_proj.tile([128, KL], f32, tag="kh2sb")
                nc.vector.tensor_copy(kh2[:], kh2_ps[:])

            # --- features: F_m (128 x 1024 bf16), G_m (128 x KL bf16) ---
            Fm, Gm = [], []
            for m in range(M):
                s_m = float(omegas[m] / (2 * PI))
                u = pool_scr.tile([128, 1024], f32, tag="uq")
                nc.vector.tensor_scalar(u[:], qh2[:], s_m, ph_q, ALU.mult, ALU.add)
                t = pool_scr.tile([128, 1024], f32, tag="tq")
                nc.vector.tensor_scalar(t[:], u[:], MAGIC, None, ALU.add)
                r = pool_scr.tile([128, 1024], f32, tag="rq")
                nc.vector.scalar_tensor_tensor(r[:], t[:], -MAGIC, u[:],
                                               ALU.add, ALU.subtract)
                F = pool_feat.tile([128, 1024], bf16, tag=f"F{m}")
                nc.scalar.activation(F[:], r[:], AF.Sin, scale=2 * PI)
                Fm.append(F)

                uk = pool_scr.tile([128, KL], f32, tag="uk")
                nc.vector.tensor_scalar(uk[:], kh2[:], s_m, ph_k, ALU.mult, ALU.add)
                tk = pool_scr.tile([128, KL], f32, tag="tk")
                nc.vector.tensor_scalar(tk[:], uk[:], MAGIC, None, ALU.add)
                rk = pool_scr.tile([128, KL], f32, tag="rk")
                nc.vector.scalar_tensor_tensor(rk[:], tk[:], -MAGIC, uk[:],
                                               ALU.add, ALU.subtract)
                graw = pool_scr.tile([128, KL], bf16, tag="graw")
                nc.scalar.activation(graw[:], rk[:], AF.Sin, scale=2 * PI)
                G = pool_feat.tile([128, KL], bf16, tag=f"G{m}")
                nc.vector.tensor_scalar(G[:], graw[:], float(coefs[m]), bwv,
                                        ALU.mult, ALU.mult)
                Gm.append(G)

            # --- scores per k-chunk -> masked exp (E kept in SBUF) ---
            Ets = []
            with (
                tc.tile_pool(name="ps_sc", bufs=2, space="PSUM") as ps_sc,
                tc.tile_pool(name="ps_num", bufs=2, space="PSUM") as ps_num,
            ):
                for c in range(kmax):
                    sc = ps_sc.tile([128, 1024], f32, tag="sc")
                    for m in range(M):
                        gsl = Gm[m][:, c * 128:(c + 1) * 128]
                        for s in range(2):
                            fsl = slice(s * 512, (s + 1) * 512)
                            nc.tensor.matmul(sc[:, fsl], gsl, Fm[m][:, fsl],
                                             start=(m == 0), stop=(m == M - 1))
                    E = pool_e.tile([128, 1024], f32, tag=f"E{c}")
                    nc.scalar.activation(E[:], sc[:], AF.Exp,
                                         bias=stage_a[:, mb0 + c:mb0 + c + 1])
                    Ets.append(E)

                # --- attn @ [V|1] per q-tile, normalize on the fly ---
                out_sb = pool_o.tile([128, 8 * 256], f32, tag="osb")
                for qt in range(8):
                    num = ps_num.tile([128, 260], f32, tag="num")
                    for c in range(kmax):
                        nc.tensor.matmul(num[:], Ets[c][:, qt * 128:(qt + 1) * 128],
                                         stage_b[:, c * 260:c * 260 + 260],
                                         start=(c == 0), stop=(c == kmax - 1))
                    rec = pool_scr.tile([128, 1], f32, tag="rec")
                    nc.vector.reciprocal(rec[:], num[:, 256:257])
                    nc.vector.tensor_scalar(out_sb[:, qt * 256:(qt + 1) * 256],
                                            num[:, 0:256], rec, None, ALU.mult)

                nc.gpsimd.dma_start(out=out_d[:], in_=out_sb[:])
    nc.compile()
    return nc


def _prep_core_inputs(b, Q, K, V, Wq, Wk, Wv, vl, kmax, omegas, coefs):
    KL = kmax * 128
    f = np.float32

    wq2 = np.concatenate([Wq, Wq], axis=1).astype(f)          # (256,128)
    wk2 = np.concatenate([Wk, Wk], axis=1).astype(f)
    ph_q = np.zeros((128, 1), f); ph_q[64:, 0] = 0.25          # F=[sin;cos]
    ph_k = np.zeros((128, 1), f); ph_k[:64, 0] = 0.25          # G=[cos;sin]
    bwv = np.concatenate([Wv, Wv]).astype(f).reshape(128, 1)
    karange = np.arange(KL).reshape(kmax, 128)
    maskb = np.where(karange < vl, 0.0, MASK_NEG).astype(f).T  # (128,kmax)

    QT = np.ascontiguousarray(Q.T.astype(f))                   # (256,1024)
    qtb = np.concatenate([QT[:128], QT[128:]], axis=1)         # (128,2048)
    KT = np.ascontiguousarray(K[:KL].T.astype(f))              # (256,KL)
    ktb = np.concatenate([KT[:128], KT[128:]], axis=1)         # (128,2*KL)

    blob_a = np.concatenate(
        [wq2[:128], wq2[128:], wk2[:128], wk2[128:],
         ph_q, ph_k, bwv, maskb, qtb, ktb], axis=1).astype(f)
    blob_a = np.ascontiguousarray(blob_a)

    vaug = np.concatenate(
        [V[:KL].astype(f), np.ones((KL, 1), f), np.zeros((KL, 3), f)], axis=1)
    blob_b = np.ascontiguousarray(
        np.concatenate([vaug[c * 128:(c + 1) * 128] for c in range(kmax)], axis=1))
    return {"blob_a": blob_a, "blob_b": blob_b}


def kernel(Q_batch, K_batch, V_batch, Wq, Wk, Wv, valid_lens):
    from concourse.bass_utils import run_bass_kernel_spmd

    Q = np.asarray(Q_batch, dtype=np.float32)
    K = np.asarray(K_batch, dtype=np.float32)
    V = np.asarray(V_batch, dtype=np.float32)
    Wq_ = np.asarray(Wq, dtype=np.float32)
    Wk_ = np.asarray(Wk, dtype=np.float32)
    Wv_ = np.asarray(Wv, dtype=np.float32)
    vls = np.asarray(valid_lens).astype(np.int64)
    assert Q.shape == (B, NQ, DQ) and K.shape == (B, NK, DK) and V.shape == (B, NK, DV)

    vls = np.clip(vls, 1, NK)
    kmax = int(np.ceil(vls.max() / 128))

    # pick fit: precomputed optimized frequencies, or refit if out of domain
    qmax = float(np.abs(Q @ Wq_).max())
    kmaxabs = float(np.abs(K @ Wk_).max())
    bound = qmax + kmaxabs
    if bound <= FIT_BOUND:
        omegas, coefs, fit_id = OMEGAS, COEFS, "std"
    else:
        omegas, coefs = _harmonic_fallback_fit(bound * 1.01)
        fit_id = f"fb{bound:.2f}"

    key = (kmax, fit_id)
    if key not in _BUILD_CACHE:
        _BUILD_CACHE[key] = _build(kmax, omegas, coefs)
    nc = _BUILD_CACHE[key]

    in_maps = [
        _prep_core_inputs(b, Q[b], K[b], V[b], Wq_, Wk_, Wv_, int(vls[b]),
                          kmax, omegas, coefs)
        for b in range(B)
    ]
    res = run_bass_kernel_spmd(nc, in_maps, core_ids=list(range(8)))

    out = np.empty((B, NQ, DV), np.float32)
    for b in range(B):
        o = res.results[b]["out"]                    # (128, 2048)
        out[b] = o.reshape(128, 8, 256).transpose(1, 0, 2).reshape(NQ, DV)
    return out


# revision 7
# speedup vs baseline: 9.2516x; 9.2516x over previous
"""Trainium2 Bass kernel for nn_AdditiveAttention (B=8, NQ=NK=1024, D=256, H=64).

Strategy
--------
Data-parallel over batch: core b owns batch element b (no collectives).

The reference computes  scores[q,k] = sum_h Wv[h] * tanh(Qh[q,h] + Kh[k,h])
with Qh = Q@Wq, Kh = K@Wk, then a masked softmax over k and attn @ V.
A direct tanh evaluation needs NQ*NK*H = 67M ScalarE ops per core (~450us).

Instead we use an optimized sine expansion (odd function):
    tanh(x) ~= sum_m b_m sin(w_m x),   max err 8e-4 on |x| <= 10.35
and the angle-addition identity
    sin(w(a+b)) = sin(wa)cos(wb) + cos(wa)sin(wb)
which turns the score computation into M rank-128 TensorE contractions with
per-side features F_m = [sin(w_m Qh); cos(w_m Qh)] (128 x NQ, bf16) and
G_m = b_m*[Wv;Wv] * [cos(w_m Kh); sin(w_m Kh)] (128 x KL, bf16).

The device Sin activation is only valid on [-pi, pi], so each feature
argument is range-reduced with the fp32 magic-constant rounding trick:
    u = x*(w/2pi) + phase;  t = u + 1.5*2^23 (rounds);  -r = (t - C) - u
    feature = sin(-2pi r) = -sin(w x + 2pi phase)
(both sides negated -> the signs cancel in the products).  u,t run on
GPSIMD, the combine (stt) on VectorE, Sin on ScalarE — spreading the
feature work across engines so TensorE stays the bottleneck.

Masking: columns k >= valid_len get exp(scores - 40) ~= 0 via a per-partition
bias on the Exp activation (k lives on partitions).  Only ceil(max(vl)/128)
k-chunks are computed at all.  The ones-column appended to V yields the
softmax denominator from the same matmul that computes attn @ V (bf16
attn weights / V, fp32 PSUM accumulation), and the output is normalized
on-device with a VectorE reciprocal + per-partition scale.

The first min(4,kmax) chunk accumulators live in PSUM across the feature
loop so TensorE consumes each harmonic's features as soon as they are
ready; the remaining chunks run as a second PE-only pass.
"""

import numpy as np

B, NQ, NK, DQ, DK, DV, H = 8, 1024, 1024, 256, 256, 256, 64
PI = float(np.pi)
MAGIC = float(1.5 * 2 ** 23)
MASK_NEG = -40.0

# M=10 frequency-optimized sine fit of tanh on [-10.35, 10.35]; max err 8.0e-4.
FIT_BOUND = 10.35
OMEGAS = [0.251318817, 0.746061764, 1.06480279, 1.33678918, 1.8444882,
          2.382742, 2.93435819, 3.49596816, 4.06399384, 4.62393325]
COEFS = [1.23949168, 0.324806206, 0.0559147295, 0.114892657, 0.0585742911,
         0.0258680335, 0.0110960367, 0.00466259993, 0.00191840823,
         0.000739195029]

_BUILD_CACHE: dict = {}


def _harmonic_fallback_fit(bound: float):
    """Numpy-only harmonic LSQ fit of tanh on [0, bound] (robustness path for
    inputs outside the precomputed fit domain)."""
    M, L = 24, bound * 1.16
    x = np.linspace(0, bound, 6001)
    om = np.arange(1, M + 1) * np.pi / L
    A = np.sin(np.outer(x, om))
    b, *_ = np.linalg.lstsq(A, np.tanh(x), rcond=None)
    return [float(v) for v in om], [float(v) for v in b]


def _build(kmax: int, omegas, coefs, repeat: int = 1):
    import concourse.bacc as bacc
    import concourse.tile as tile
    from concourse import mybir

    AF = mybir.ActivationFunctionType
    ALU = mybir.AluOpType
    f32 = mybir.dt.float32
    bf16 = mybir.dt.bfloat16

    M = len(omegas)
    KL = kmax * 128
    # blob_a: [wq2 256 | wk2 256 | ph_q 1 | ph_k 1 | bwv 1 | maskb kmax | qtb 2048]
    WA = 512 + 3 + kmax + 2048
    # blob_b: [ktb 2*KL | vaugb kmax*260]
    WB = 2 * KL + kmax * 260

    nc = bacc.Bacc("TRN2", target_bir_lowering=False, debug=False,
                   num_devices=8, num_swdge_queues=2)
    blob_a = nc.declare_dram_parameter("blob_a", [128, WA], f32, isOutput=False)
    blob_b = nc.declare_dram_parameter("blob_b", [128, WB], f32, isOutput=False)
    out_d = nc.declare_dram_parameter("out", [128, 8 * 256], f32, isOutput=True)

    NCH1 = min(3, kmax)          # chunks accumulated during the feature loop

    with tile.TileContext(nc) as tc:
        with (
            tc.tile_pool(name="io", bufs=1) as pool_io,
            tc.tile_pool(name="proj_sb", bufs=1) as pool_proj,
            tc.tile_pool(name="scratch", bufs=2) as pool_scr,
            tc.tile_pool(name="feat", bufs=1) as pool_feat,
            tc.tile_pool(name="epool", bufs=1) as pool_e,
            tc.tile_pool(name="opool", bufs=1) as pool_o,
        ):
            stage_a = pool_io.tile([128, WA], f32, tag="stage_a")
            nc.gpsimd.dma_start(out=stage_a[:], in_=blob_a[:])
            stage_b = pool_io.tile([128, WB], f32, tag="stage_b")
            nc.gpsimd.dma_start(out=stage_b[:], in_=blob_b[:])

            wq2 = [stage_a[:, 0:128], stage_a[:, 128:256]]
            wk2 = [stage_a[:, 256:384], stage_a[:, 384:512]]
            ph_q = stage_a[:, 512:513]
            ph_k = stage_a[:, 513:514]
            bwv = stage_a[:, 514:515]
            mb0 = 515
            qt0 = 515 + kmax
            qtv = [stage_a[:, qt0:qt0 + 1024], stage_a[:, qt0 + 1024:qt0 + 2048]]
            ktv = [stage_b[:, 0:KL], stage_b[:, KL:2 * KL]]
            va0 = 2 * KL

            for _rep in range(repeat):
                _build_body(nc, tc, tile, mybir, kmax, omegas, coefs,
                            stage_a, stage_b, pool_proj, pool_scr, pool_feat,
                            pool_e, pool_o, out_d, last=(_rep == repeat - 1))
    nc.compile()
    return nc


def _build_body(nc, tc, tile, mybir, kmax, omegas, coefs, stage_a, stage_b,
                pool_proj, pool_scr, pool_feat, pool_e, pool_o, out_d, last):
    AF = mybir.ActivationFunctionType
    ALU = mybir.AluOpType
    f32 = mybir.dt.float32
    bf16 = mybir.dt.bfloat16
    M = len(omegas)
    KL = kmax * 128
    NCH1 = min(3, kmax)
    if True:
        if True:
            wq2 = [stage_a[:, 0:128], stage_a[:, 128:256]]
            wk2 = [stage_a[:, 256:384], stage_a[:, 384:512]]
            ph_q = stage_a[:, 512:513]
            ph_k = stage_a[:, 513:514]
            bwv = stage_a[:, 514:515]
            mb0 = 515
            qt0 = 515 + kmax
            qtv = [stage_a[:, qt0:qt0 + 1024], stage_a[:, qt0 + 1024:qt0 + 2048]]
            ktv = [stage_b[:, 0:KL], stage_b[:, KL:2 * KL]]
            va0 = 2 * KL

            # bf16 copy of [V | 1] for the num-stage moving operand
            vaug16 = pool_proj.tile([128, kmax * 260], bf16, tag="vaug16")
            nc.vector.tensor_copy(vaug16[:], stage_b[:, va0:va0 + kmax * 260])

            # --- projections: Qh2 = [Wq|Wq]^T Q^T (128 x 1024), same for K ---
            with tc.tile_pool(name="ps_proj", bufs=1, space="PSUM") as ps_proj:
                qh2_ps = ps_proj.tile([128, 1024], f32, tag="qh2")
                for s in range(2):  # f32 moving limit 512
                    sl = slice(s * 512, (s + 1) * 512)
                    nc.tensor.matmul(qh2_ps[:, sl], wq2[0], qtv[0][:, sl],
                                     start=True, stop=False)
                    nc.tensor.matmul(qh2_ps[:, sl], wq2[1], qtv[1][:, sl],
                                     start=False, stop=True)
                kh2_ps = ps_proj.tile([128, KL], f32, tag="kh2")
                nks = (KL + 511) // 512
                for s in range(nks):
                    sl = slice(s * 512, min((s + 1) * 512, KL))
                    nc.tensor.matmul(kh2_ps[:, sl], wk2[0], ktv[0][:, sl],
                                     start=True, stop=False)
                    nc.tensor.matmul(kh2_ps[:, sl], wk2[1], ktv[1][:, sl],
                                     start=False, stop=True)

                qh2 = pool_proj.tile([128, 1024], f32, tag="qh2sb")
                nc.vector.tensor_copy(qh2[:], qh2_ps[:])
                kh2 = pool_proj.tile([128, KL], f32, tag="kh2sb")
                nc.vector.tensor_copy(kh2[:], kh2_ps[:])

            Ets = [None] * kmax
            with (
                tc.tile_pool(name="ps_sc", bufs=3, space="PSUM") as ps_sc,
                tc.tile_pool(name="ps_num", bufs=2, space="PSUM") as ps_num,
            ):
                # --- feature loop, fused with the first NCH1 chunks' matmuls
                sc1 = []
                for _c in range(NCH1):
                    sct = ps_sc.tile([128, 1024], f32, tag="sc", name=f"sc1_{_c}")
                    sc1.append(sct)
                Fm, Gm = [], []
                for m in range(M):
                    s_m = float(omegas[m] / (2 * PI))
                    u = pool_scr.tile([128, 1024], f32, tag="uq")
                    nc.gpsimd.tensor_scalar(u[:], qh2[:], s_m, ph_q, ALU.mult, ALU.add)
                    t = pool_scr.tile([128, 1024], f32, tag="tq")
                    nc.gpsimd.tensor_scalar(t[:], u[:], MAGIC, None, ALU.add)
                    r = pool_scr.tile([128, 1024], f32, tag="rq")
                    nc.vector.scalar_tensor_tensor(r[:], t[:], -MAGIC, u[:],
                                                   ALU.add, ALU.subtract)
                    F = pool_feat.tile([128, 1024], bf16, tag=f"F{m}")
                    nc.scalar.activation(F[:], r[:], AF.Sin, scale=2 * PI)
                    Fm.append(F)

                    uk = pool_scr.tile([128, KL], f32, tag="uk")
                    nc.gpsimd.tensor_scalar(uk[:], kh2[:], s_m, ph_k, ALU.mult, ALU.add)
                    tk = pool_scr.tile([128, KL], f32, tag="tk")
                    nc.gpsimd.tensor_scalar(tk[:], uk[:], MAGIC, None, ALU.add)
                    rk = pool_scr.tile([128, KL], f32, tag="rk")
                    nc.vector.scalar_tensor_tensor(rk[:], tk[:], -MAGIC, uk[:],
                                                   ALU.add, ALU.subtract)
                    graw = pool_scr.tile([128, KL], bf16, tag="graw")
                    nc.scalar.activation(graw[:], rk[:], AF.Sin, scale=2 * PI)
                    G = pool_feat.tile([128, KL], bf16, tag=f"G{m}")
                    nc.vector.tensor_scalar(G[:], graw[:], float(coefs[m]), bwv,
                                            ALU.mult, ALU.mult)
                    Gm.append(G)

                    for c in range(NCH1):
                        gsl = G[:, c * 128:(c + 1) * 128]
                        for s in range(2):
                            fsl = slice(s * 512, (s + 1) * 512)
                            nc.tensor.matmul(sc1[c][:, fsl], gsl, F[:, fsl],
                                             start=(m == 0), stop=(m == M - 1))

                for c in range(NCH1):
                    E = pool_e.tile([128, 1024], bf16, tag=f"E{c}")
                    nc.scalar.activation(E[:], sc1[c][:], AF.Exp,
                                         bias=stage_a[:, mb0 + c:mb0 + c + 1])
                    Ets[c] = E

                # --- remaining chunks: PE-only pass over stashed features ---
                for c in range(NCH1, kmax):
                    sc = ps_sc.tile([128, 1024], f32, tag="sc")
                    for m in range(M):
                        gsl = Gm[m][:, c * 128:(c + 1) * 128]
                        for s in range(2):
                            fsl = slice(s * 512, (s + 1) * 512)
                            nc.tensor.matmul(sc[:, fsl], gsl, Fm[m][:, fsl],
                                             start=(m == 0), stop=(m == M - 1))
                    E = pool_e.tile([128, 1024], bf16, tag=f"E{c}")
                    nc.scalar.activation(E[:], sc[:], AF.Exp,
                                         bias=stage_a[:, mb0 + c:mb0 + c + 1])
                    Ets[c] = E

                # --- attn @ [V|1] per q-tile, normalize on the fly ---
                out_sb = pool_o.tile([128, 8 * 256], f32, tag="osb")
                for qt in range(8):
                    num = ps_num.tile([128, 260], f32, tag="num")
                    for c in range(kmax):
                        nc.tensor.matmul(num[:], Ets[c][:, qt * 128:(qt + 1) * 128],
                                         vaug16[:, c * 260:c * 260 + 260],
                                         start=(c == 0), stop=(c == kmax - 1))
                    rec = pool_scr.tile([128, 1], f32, tag="rec")
                    nc.vector.reciprocal(rec[:], num[:, 256:257])
                    nc.vector.tensor_scalar(out_sb[:, qt * 256:(qt + 1) * 256],
                                            num[:, 0:256], rec, None, ALU.mult)

                if last:
                    nc.gpsimd.dma_start(out=out_d[:], in_=out_sb[:])


def _prep_core_inputs(b, Q, K, V, Wq, Wk, Wv, vl, kmax, omegas, coefs):
    KL = kmax * 128
    f = np.float32

    wq2 = np.concatenate([Wq, Wq], axis=1).astype(f)          # (256,128)
    wk2 = np.concatenate([Wk, Wk], axis=1).astype(f)
    ph_q = np.zeros((128, 1), f); ph_q[64:, 0] = 0.25          # F=[sin;cos]
    ph_k = np.zeros((128, 1), f); ph_k[:64, 0] = 0.25          # G=[cos;sin]
    bwv = np.concatenate([Wv, Wv]).astype(f).reshape(128, 1)
    karange = np.arange(KL).reshape(kmax, 128)
    maskb = np.where(karange < vl, 0.0, MASK_NEG).astype(f).T  # (128,kmax)

    QT = np.ascontiguousarray(Q.T.astype(f))                   # (256,1024)
    qtb = np.concatenate([QT[:128], QT[128:]], axis=1)         # (128,2048)
    KT = np.ascontiguousarray(K[:KL].T.astype(f))              # (256,KL)
    ktb = np.concatenate([KT[:128], KT[128:]], axis=1)         # (128,2*KL)

    blob_a = np.ascontiguousarray(np.concatenate(
        [wq2[:128], wq2[128:], wk2[:128], wk2[128:],
         ph_q, ph_k, bwv, maskb, qtb], axis=1).astype(f))

    vaug = np.concatenate(
        [V[:KL].astype(f), np.ones((KL, 1), f), np.zeros((KL, 3), f)], axis=1)
    vaugb = np.concatenate([vaug[c * 128:(c + 1) * 128] for c in range(kmax)], axis=1)
    blob_b = np.ascontiguousarray(np.concatenate([ktb, vaugb], axis=1).astype(f))
    return {"blob_a": blob_a, "blob_b": blob_b}


def kernel(Q_batch, K_batch, V_batch, Wq, Wk, Wv, valid_lens):
    from concourse.bass_utils import run_bass_kernel_spmd

    Q = np.asarray(Q_batch, dtype=np.float32)
    K = np.asarray(K_batch, dtype=np.float32)
    V = np.asarray(V_batch, dtype=np.float32)
    Wq_ = np.asarray(Wq, dtype=np.float32)
    Wk_ = np.asarray(Wk, dtype=np.float32)
    Wv_ = np.asarray(Wv, dtype=np.float32)
    vls = np.asarray(valid_lens).astype(np.int64)
    assert Q.shape == (B, NQ, DQ) and K.shape == (B, NK, DK) and V.shape == (B, NK, DV)

    vls = np.clip(vls, 1, NK)
    kmax = int(np.ceil(vls.max() / 128))

    # pick fit: precomputed optimized frequencies, or refit if out of domain
    qmax = float(np.abs(Q @ Wq_).max())
    kmaxabs = float(np.abs(K @ Wk_).max())
    bound = qmax + kmaxabs
    if bound <= FIT_BOUND:
        omegas, coefs, fit_id = OMEGAS, COEFS, "std"
    else:
        omegas, coefs = _harmonic_fallback_fit(bound * 1.01)
        fit_id = f"fb{bound:.2f}"

    key = (kmax, fit_id)
    if key not in _BUILD_CACHE:
        _BUILD_CACHE[key] = _build(kmax, omegas, coefs)
    nc = _BUILD_CACHE[key]

    in_maps = [
        _prep_core_inputs(b, Q[b], K[b], V[b], Wq_, Wk_, Wv_, int(vls[b]),
                          kmax, omegas, coefs)
        for b in range(B)
    ]
    res = run_bass_kernel_spmd(nc, in_maps, core_ids=list(range(8)))

    out = np.empty((B, NQ, DV), np.float32)
    for b in range(B):
        o = res.results[b]["out"]                    # (128, 2048)
        out[b] = o.reshape(128, 8, 256).transpose(1, 0, 2).reshape(NQ, DV)
    return out


# revision 8
# speedup vs baseline: 69.7608x; 7.5404x over previous
"""Trainium2 Bass kernel for nn_AdditiveAttention (B=8, NQ=NK=1024, D=256, H=64).

Strategy
--------
Data-parallel over batch: core b owns batch element b (no collectives).

The reference computes  scores[q,k] = sum_h Wv[h] * tanh(Qh[q,h] + Kh[k,h])
with Qh = Q@Wq, Kh = K@Wk, then a masked softmax over k and attn @ V.
A direct tanh evaluation needs NQ*NK*H = 67M ScalarE ops per core (~450us).

Instead we use an optimized sine expansion (odd function):
    tanh(x) ~= sum_m b_m sin(w_m x),   max err 8e-4 on |x| <= 10.35
and the angle-addition identity
    sin(w(a+b)) = sin(wa)cos(wb) + cos(wa)sin(wb)
which turns the score computation into M rank-128 TensorE contractions with
per-side features F_m = [sin(w_m Qh); cos(w_m Qh)] (128 x NQ, bf16) and
G_m = b_m*[Wv;Wv] * [cos(w_m Kh); sin(w_m Kh)] (128 x KL, bf16).

The device Sin activation is only valid on [-pi, pi], so each feature
argument is range-reduced with the fp32 magic-constant rounding trick:
    u = x*(w/2pi) + phase;  t = u + 1.5*2^23 (rounds);  -r = (t - C) - u
    feature = sin(-2pi r) = -sin(w x + 2pi phase)
(both sides negated -> the signs cancel in the products).  u,t run on
GPSIMD, the combine (stt) on VectorE, Sin on ScalarE — spreading the
feature work across engines so TensorE stays the bottleneck.

Masking: columns k >= valid_len get exp(scores - 40) ~= 0 via a per-partition
bias on the Exp activation (k lives on partitions).  Only ceil(max(vl)/128)
k-chunks are computed at all.  The ones-column appended to V yields the
softmax denominator from the same matmul that computes attn @ V (bf16
attn weights / V, fp32 PSUM accumulation), and the output is normalized
on-device with a VectorE reciprocal + per-partition scale.

The first min(4,kmax) chunk accumulators live in PSUM across the feature
loop so TensorE consumes each harmonic's features as soon as they are
ready; the remaining chunks run as a second PE-only pass.
"""

import numpy as np

B, NQ, NK, DQ, DK, DV, H = 8, 1024, 1024, 256, 256, 256, 64
PI = float(np.pi)
MAGIC = float(1.5 * 2 ** 23)
MASK_NEG = -40.0

# M=10 frequency-optimized sine fit of tanh on [-10.35, 10.35]; max err 8.0e-4.
FIT_BOUND = 10.35
OMEGAS = [0.251318817, 0.746061764, 1.06480279, 1.33678918, 1.8444882,
          2.382742, 2.93435819, 3.49596816, 4.06399384, 4.62393325]
COEFS = [1.23949168, 0.324806206, 0.0559147295, 0.114892657, 0.0585742911,
         0.0258680335, 0.0110960367, 0.00466259993, 0.00191840823,
         0.000739195029]

_BUILD_CACHE: dict = {}


def _harmonic_fallback_fit(bound: float):
    """Numpy-only harmonic LSQ fit of tanh on [0, bound] (robustness path for
    inputs outside the precomputed fit domain)."""
    M, L = 24, bound * 1.16
    x = np.linspace(0, bound, 6001)
    om = np.arange(1, M + 1) * np.pi / L
    A = np.sin(np.outer(x, om))
    b, *_ = np.linalg.lstsq(A, np.tanh(x), rcond=None)
    return [float(v) for v in om], [float(v) for v in b]


def _build(kmax: int, omegas, coefs, repeat: int = 1):
    import concourse.bacc as bacc
    import concourse.tile as tile
    from concourse import mybir

    AF = mybir.ActivationFunctionType
    ALU = mybir.AluOpType
    f32 = mybir.dt.float32
    bf16 = mybir.dt.bfloat16

    M = len(omegas)
    KL = kmax * 128
    # blob_a: [wq2 256 | wk2 256 | ph_q 1 | ph_k 1 | bwv 1 | maskb kmax | qtb 2048]
    WA = 512 + 3 + kmax + 2048
    # blob_b: [ktb 2*KL | vaugb kmax*260]
    WB = 2 * KL + kmax * 260

    nc = bacc.Bacc("TRN2", target_bir_lowering=False, debug=False,
                   num_devices=8, num_swdge_queues=2)
    blob_a = nc.declare_dram_parameter("blob_a", [128, WA], f32, isOutput=False)
    blob_b = nc.declare_dram_parameter("blob_b", [128, WB], f32, isOutput=False)
    out_d = nc.declare_dram_parameter("out", [128, 8 * 256], f32, isOutput=True)

    NCH1 = min(3, kmax)          # chunks accumulated during the feature loop

    with tile.TileContext(nc) as tc:
        with (
            tc.tile_pool(name="io", bufs=1) as pool_io,
            tc.tile_pool(name="proj_sb", bufs=1) as pool_proj,
            tc.tile_pool(name="scratch", bufs=2) as pool_scr,
            tc.tile_pool(name="feat", bufs=1) as pool_feat,
            tc.tile_pool(name="epool", bufs=1) as pool_e,
            tc.tile_pool(name="opool", bufs=1) as pool_o,
        ):
            stage_a = pool_io.tile([128, WA], f32, tag="stage_a")
            nc.gpsimd.dma_start(out=stage_a[:], in_=blob_a[:])
            stage_b = pool_io.tile([128, WB], f32, tag="stage_b")
            nc.gpsimd.dma_start(out=stage_b[:], in_=blob_b[:])

            wq2 = [stage_a[:, 0:128], stage_a[:, 128:256]]
            wk2 = [stage_a[:, 256:384], stage_a[:, 384:512]]
            ph_q = stage_a[:, 512:513]
            ph_k = stage_a[:, 513:514]
            bwv = stage_a[:, 514:515]
            mb0 = 515
            qt0 = 515 + kmax
            qtv = [stage_a[:, qt0:qt0 + 1024], stage_a[:, qt0 + 1024:qt0 + 2048]]
            ktv = [stage_b[:, 0:KL], stage_b[:, KL:2 * KL]]
            va0 = 2 * KL

            for _rep in range(repeat):
                _build_body(nc, tc, tile, mybir, kmax, omegas, coefs,
                            stage_a, stage_b, pool_proj, pool_scr, pool_feat,
                            pool_e, pool_o, out_d, last=(_rep == repeat - 1))
    nc.compile()
    return nc


GPSIMD_FEATS = True


def _build_body(nc, tc, tile, mybir, kmax, omegas, coefs, stage_a, stage_b,
                pool_proj, pool_scr, pool_feat, pool_e, pool_o, out_d, last):
    AF = mybir.ActivationFunctionType
    ALU = mybir.AluOpType
    f32 = mybir.dt.float32
    bf16 = mybir.dt.bfloat16
    M = len(omegas)
    KL = kmax * 128
    NCH1 = min(3, kmax)
    if True:
        if True:
            wq2 = [stage_a[:, 0:128], stage_a[:, 128:256]]
            wk2 = [stage_a[:, 256:384], stage_a[:, 384:512]]
            ph_q = stage_a[:, 512:513]
            ph_k = stage_a[:, 513:514]
            bwv = stage_a[:, 514:515]
            mb0 = 515
            qt0 = 515 + kmax
            qtv = [stage_a[:, qt0:qt0 + 1024], stage_a[:, qt0 + 1024:qt0 + 2048]]
            ktv = [stage_b[:, 0:KL], stage_b[:, KL:2 * KL]]
            va0 = 2 * KL

            # bf16 copy of [V | 1] for the num-stage moving operand
            vaug16 = pool_proj.tile([128, kmax * 260], bf16, tag="vaug16")
            nc.vector.tensor_copy(vaug16[:], stage_b[:, va0:va0 + kmax * 260])

            # --- projections: Qh2 = [Wq|Wq]^T Q^T (128 x 1024), same for K ---
            with tc.tile_pool(name="ps_proj", bufs=1, space="PSUM") as ps_proj:
                qh2_ps = ps_proj.tile([128, 1024], f32, tag="qh2")
                for s in range(2):  # f32 moving limit 512
                    sl = slice(s * 512, (s + 1) * 512)
                    nc.tensor.matmul(qh2_ps[:, sl], wq2[0], qtv[0][:, sl],
                                     start=True, stop=False)
                    nc.tensor.matmul(qh2_ps[:, sl], wq2[1], qtv[1][:, sl],
                                     start=False, stop=True)
                kh2_ps = ps_proj.tile([128, KL], f32, tag="kh2")
                nks = (KL + 511) // 512
                for s in range(nks):
                    sl = slice(s * 512, min((s + 1) * 512, KL))
                    nc.tensor.matmul(kh2_ps[:, sl], wk2[0], ktv[0][:, sl],
                                     start=True, stop=False)
                    nc.tensor.matmul(kh2_ps[:, sl], wk2[1], ktv[1][:, sl],
                                     start=False, stop=True)

                qh2 = pool_proj.tile([128, 1024], f32, tag="qh2sb")
                nc.vector.tensor_copy(qh2[:], qh2_ps[:])
                kh2 = pool_proj.tile([128, KL], f32, tag="kh2sb")
                nc.vector.tensor_copy(kh2[:], kh2_ps[:])

            Ets = [None] * kmax
            with (
                tc.tile_pool(name="ps_sc", bufs=3, space="PSUM") as ps_sc,
                tc.tile_pool(name="ps_num", bufs=2, space="PSUM") as ps_num,
            ):
                # --- feature loop, fused with the first NCH1 chunks' matmuls
                sc1 = []
                for _c in range(NCH1):
                    sct = ps_sc.tile([128, 1024], f32, tag="sc", name=f"sc1_{_c}")
                    sc1.append(sct)
                Fm, Gm = [], []
                for m in range(M):
                    s_m = float(omegas[m] / (2 * PI))
                    fe = nc.gpsimd if GPSIMD_FEATS else nc.vector
                    u = pool_scr.tile([128, 1024], f32, tag="uq")
                    fe.tensor_scalar(u[:], qh2[:], s_m, ph_q, ALU.mult, ALU.add)
                    t = pool_scr.tile([128, 1024], f32, tag="tq")
                    fe.tensor_scalar(t[:], u[:], MAGIC, None, ALU.add)
                    r = pool_scr.tile([128, 1024], f32, tag="rq")
                    nc.vector.scalar_tensor_tensor(r[:], t[:], -MAGIC, u[:],
                                                   ALU.add, ALU.subtract)
                    F = pool_feat.tile([128, 1024], bf16, tag=f"F{m}")
                    nc.scalar.activation(F[:], r[:], AF.Sin, scale=2 * PI)
                    Fm.append(F)

                    uk = pool_scr.tile([128, KL], f32, tag="uk")
                    fe.tensor_scalar(uk[:], kh2[:], s_m, ph_k, ALU.mult, ALU.add)
                    tk = pool_scr.tile([128, KL], f32, tag="tk")
                    fe.tensor_scalar(tk[:], uk[:], MAGIC, None, ALU.add)
                    rk = pool_scr.tile([128, KL], f32, tag="rk")
                    nc.vector.scalar_tensor_tensor(rk[:], tk[:], -MAGIC, uk[:],
                                                   ALU.add, ALU.subtract)
                    graw = pool_scr.tile([128, KL], bf16, tag="graw")
                    nc.scalar.activation(graw[:], rk[:], AF.Sin, scale=2 * PI)
                    G = pool_feat.tile([128, KL], bf16, tag=f"G{m}")
                    nc.vector.tensor_scalar(G[:], graw[:], float(coefs[m]), bwv,
                                            ALU.mult, ALU.mult)
                    Gm.append(G)

                    for c in range(NCH1):
                        gsl = G[:, c * 128:(c + 1) * 128]
                        for s in range(2):
                            fsl = slice(s * 512, (s + 1) * 512)
                            nc.tensor.matmul(sc1[c][:, fsl], gsl, F[:, fsl],
                                             start=(m == 0), stop=(m == M - 1))

                for c in range(NCH1):
                    E = pool_e.tile([128, 1024], bf16, tag=f"E{c}")
                    nc.scalar.activation(E[:], sc1[c][:], AF.Exp,
                                         bias=stage_a[:, mb0 + c:mb0 + c + 1])
                    Ets[c] = E

                # --- remaining chunks: PE-only pass over stashed features ---
                for c in range(NCH1, kmax):
                    sc = ps_sc.tile([128, 1024], f32, tag="sc")
                    for m in range(M):
                        gsl = Gm[m][:, c * 128:(c + 1) * 128]
                        for s in range(2):
                            fsl = slice(s * 512, (s + 1) * 512)
                            nc.tensor.matmul(sc[:, fsl], gsl, Fm[m][:, fsl],
                                             start=(m == 0), stop=(m == M - 1))
                    E = pool_e.tile([128, 1024], bf16, tag=f"E{c}")
                    nc.scalar.activation(E[:], sc[:], AF.Exp,
                                         bias=stage_a[:, mb0 + c:mb0 + c + 1])
                    Ets[c] = E

                # --- attn @ [V|1] per q-tile, normalize on the fly ---
                out_sb = pool_o.tile([128, 8 * 256], f32, tag="osb")
                for qt in range(8):
                    num = ps_num.tile([128, 260], f32, tag="num")
                    for c in range(kmax):
                        nc.tensor.matmul(num[:], Ets[c][:, qt * 128:(qt + 1) * 128],
                                         vaug16[:, c * 260:c * 260 + 260],
                                         start=(c == 0), stop=(c == kmax - 1))
                    rec = pool_scr.tile([128, 1], f32, tag="rec")
                    nc.vector.reciprocal(rec[:], num[:, 256:257])
                    nc.vector.tensor_scalar(out_sb[:, qt * 256:(qt + 1) * 256],
                                            num[:, 0:256], rec, None, ALU.mult)

                if last:
                    nc.gpsimd.dma_start(out=out_d[:], in_=out_sb[:])


def _prep_core_inputs(b, Q, K, V, Wq, Wk, Wv, vl, kmax, omegas, coefs):
    KL = kmax * 128
    f = np.float32

    wq2 = np.concatenate([Wq, Wq], axis=1).astype(f)          # (256,128)
    wk2 = np.concatenate([Wk, Wk], axis=1).astype(f)
    ph_q = np.zeros((128, 1), f); ph_q[64:, 0] = 0.25          # F=[sin;cos]
    ph_k = np.zeros((128, 1), f); ph_k[:64, 0] = 0.25          # G=[cos;sin]
    bwv = np.concatenate([Wv, Wv]).astype(f).reshape(128, 1)
    karange = np.arange(KL).reshape(kmax, 128)
    maskb = np.where(karange < vl, 0.0, MASK_NEG).astype(f).T  # (128,kmax)

    QT = np.ascontiguousarray(Q.T.astype(f))                   # (256,1024)
    qtb = np.concatenate([QT[:128], QT[128:]], axis=1)         # (128,2048)
    KT = np.ascontiguousarray(K[:KL].T.astype(f))              # (256,KL)
    ktb = np.concatenate([KT[:128], KT[128:]], axis=1)         # (128,2*KL)

    blob_a = np.ascontiguousarray(np.concatenate(
        [wq2[:128], wq2[128:], wk2[:128], wk2[128:],
         ph_q, ph_k, bwv, maskb, qtb], axis=1).astype(f))

    vaug = np.concatenate(
        [V[:KL].astype(f), np.ones((KL, 1), f), np.zeros((KL, 3), f)], axis=1)
    vaugb = np.concatenate([vaug[c * 128:(c + 1) * 128] for c in range(kmax)], axis=1)
    blob_b = np.ascontiguousarray(np.concatenate([ktb, vaugb], axis=1).astype(f))
    return {"blob_a": blob_a, "blob_b": blob_b}


def kernel(Q_batch, K_batch, V_batch, Wq, Wk, Wv, valid_lens):
    from concourse.bass_utils import run_bass_kernel_spmd

    Q = np.asarray(Q_batch, dtype=np.float32)
    K = np.asarray(K_batch, dtype=np.float32)
    V = np.asarray(V_batch, dtype=np.float32)
    Wq_ = np.asarray(Wq, dtype=np.float32)
    Wk_ = np.asarray(Wk, dtype=np.float32)
    Wv_ = np.asarray(Wv, dtype=np.float32)
    vls = np.asarray(valid_lens).astype(np.int64)
    assert Q.shape == (B, NQ, DQ) and K.shape == (B, NK, DK) and V.shape == (B, NK, DV)

    vls = np.clip(vls, 1, NK)
    kmax = int(np.ceil(vls.max() / 128))

    # pick fit: precomputed optimized frequencies, or refit if out of domain
    qmax = float(np.abs(Q @ Wq_).max())
    kmaxabs = float(np.abs(K @ Wk_).max())
    bound = qmax + kmaxabs
    if bound <= FIT_BOUND:
        omegas, coefs, fit_id = OMEGAS, COEFS, "std"
    else:
        omegas, coefs = _harmonic_fallback_fit(bound * 1.01)
        fit_id = f"fb{bound:.2f}"

    key = (kmax, fit_id)
    if key not in _BUILD_CACHE:
        _BUILD_CACHE[key] = _build(kmax, omegas, coefs)
    nc = _BUILD_CACHE[key]

    in_maps = [
        _prep_core_inputs(b, Q[b], K[b], V[b], Wq_, Wk_, Wv_, int(vls[b]),
                          kmax, omegas, coefs)
        for b in range(B)
    ]
    res = run_bass_kernel_spmd(nc, in_maps, core_ids=list(range(8)))

    out = np.empty((B, NQ, DV), np.float32)
    for b in range(B):
        o = res.results[b]["out"]                    # (128, 2048)
        out[b] = o.reshape(128, 8, 256).transpose(1, 0, 2).reshape(NQ, DV)
    return out
